# revision 25
# baseline (speedup 1.0000x reference)
"""Causal multi-head self-attention on 8 Trainium2 NeuronCores.

Sharding: 8 cores = (batch b in 0..3) x (head-half hh in 0..1).
Each core computes Q/K/V projections for its 1024-wide slice of the hidden
dim (8 of 16 heads), causal attention for those heads, and the partial
output projection against the matching 1024 rows of Wo^T.  The two partial
outputs per batch are summed on the host at gather time (the tensor-parallel
all-reduce).

Schedule notes:
- PE warm-up matmuls on a zeroed dummy tile start the HAM clock ramp while
  the first DMAs land; the Q-projection group order interleaves the two
  s-halves ((0,0),(1,0),(2,0),(0,1),(3,0),(1,1),...) so the first groups
  only depend on the first 4 MB of x^T and the weight stream stays just
  ahead of the PE with a 5-tile window and no re-streaming.
- The softmax normalization is folded into the P-"transpose" on the PE: it
  is issued as a regular matmul against diag(1/r) (built by one cheap DVE
  multiply of the identity tile by the accumulated exp sums), so the
  transposed P comes out pre-normalized and the [128,S]-wide DVE scale pass
  disappears.  (Hardware transpose mode is a pure permutation datapath and
  cannot scale, so this must be a normal matmul with f32 PSUM out.)
- Attention runs as a rolling software pipeline: scores for the even q-tile
  of a pair are issued 3 heads ahead, the odd tile 2 heads ahead of the
  transpose+PV that consumes them, with V-projection s-tiles and output-
  projection chunks placed as PE fillers, so the PE never waits on the
  ACT/DVE softmax chain.

Problem constants (hardcoded): B=4, S=1024, D=2048, H=16, DH=128,
softmax scale = sqrt(DH) (faithful to the reference, which multiplies
scores by head_dim**0.5).
"""

import numpy as np

import concourse.bass as bass
import concourse.tile as tile
from concourse import bacc, mybir
from concourse.bass_utils import run_bass_kernel_spmd

B, S, D = 4, 1024, 2048
H = 16
DH = 128
SCALE = float(DH) ** 0.5
HL = 8          # heads per core
E = HL * DH     # 1024: per-core slice of hidden dim
KO = D // 128   # 16 k-subtiles for d-contraction
ST = S // 128   # 8 sequence tiles
F32 = mybir.dt.float32
F32R = mybir.dt.float32r
BF16 = mybir.dt.bfloat16
NEG = -1.0e30

# Q-projection group order: sc=1 groups trail their e by 3 slots so the
# second s-half of x^T has time to stream in behind the first.
QGROUPS = [(0, 0), (1, 0), (2, 0), (0, 1), (1, 1), (3, 0), (2, 1), (4, 0),
           (3, 1), (5, 0), (4, 1), (6, 0), (5, 1), (7, 0), (6, 1), (7, 1)]


def build_nc():
    nc = bacc.Bacc("TRN2", target_bir_lowering=False, debug=False, num_devices=8)

    # x[b]^T host-pre-tiled to [sc-half, p, ko, c]: each 1 MB DMA chunk is
    # an 8 KB contiguous run per partition.
    xt = nc.dram_tensor("xt", [2, 128, KO, 512], F32R, kind="ExternalInput")
    # Wq/Wk for this head-half, host-pre-tiled to [e-tile, p, ko, c] so each
    # 1 MB weight tile is one contiguous 8 KB run per partition for the DMA.
    wqt = nc.dram_tensor("wqt", [HL, 128, KO, 128], F32R, kind="ExternalInput")
    wkt = nc.dram_tensor("wkt", [HL, 128, KO, 128], F32R, kind="ExternalInput")
    xtbf = nc.dram_tensor("xtbf", [D, S], BF16, kind="ExternalInput")  # x[b].T bf16
    wvt = nc.dram_tensor("wvt", [D, E], BF16, kind="ExternalInput")
    wot = nc.dram_tensor("wot", [E, D], BF16, kind="ExternalInput")     # Wo[:, slice].T
    maskd = nc.dram_tensor("maskd", [128, 128], F32, kind="ExternalInput")
    identd = nc.dram_tensor("identd", [128, 128], BF16, kind="ExternalInput")
    out = nc.dram_tensor("out", [S, D], F32, kind="ExternalOutput")

    xt_r = xt.ap().rearrange("sc p ko c -> p sc ko c")   # [128, 2, 16, 512]
    xtbf_r = xtbf.ap().rearrange("(ko p) s -> p ko s", p=128)
    wqt_r = wqt.ap().rearrange("e p ko c -> p e ko c")  # [128, 8, 16, 128]
    wkt_r = wkt.ap().rearrange("e p ko c -> p e ko c")
    wvt_r = wvt.ap().rearrange("(ko p) e -> p ko e", p=128)
    wot_r = wot.ap().rearrange("(eo p) o -> p eo o", p=128)  # [128, 8, 2048]
    out_r = out.ap().rearrange("(so p) o -> p so o", p=128)  # [128, 8, 2048]

    with tile.TileContext(nc) as tc:
        # PSUM pools: 8 banks total on the core.
        pp = tc.alloc_tile_pool(name="pp", bufs=2, space="PSUM")      # proj/outproj
        ps_s = tc.alloc_tile_pool(name="ps_s", bufs=3, space="PSUM")  # scores
        ps_t = tc.alloc_tile_pool(name="ps_t", bufs=2, space="PSUM")  # transposes
        ps_c = tc.alloc_tile_pool(name="ps_c", bufs=1, space="PSUM")  # ctx accum

        # Long-lived SBUF (stack-allocated first).
        persist = tc.alloc_tile_pool(name="persist", bufs=1)
        mask_sb = persist.tile([128, 128], F32)
        ident_sb = persist.tile([128, 128], BF16)
        dummy_sb = persist.tile([128, 512], BF16)
        nc.vector.memset(dummy_sb[:], 0.0)

        def dummy_mms(n):
            """HAM-keepalive: n dense throwaway matmuls with no DMA deps."""
            wps = pp.tile([128, 512], F32, tag="pp")
            for i in range(n):
                nc.tensor.matmul(
                    wps[:], dummy_sb[:, 0:128], dummy_sb[:],
                    start=(i == 0), stop=(i == n - 1),
                )
            dout = smalls.tile([128, 1], F32, tag="dout")
            nc.vector.tensor_copy(dout[:], wps[:, 0:1])

        qt_sb = persist.tile([128, HL, S], F32R)   # QT: [dh, head, s]
        kt_sb = persist.tile([128, HL, S], F32R)

        smalls = tc.alloc_tile_pool(name="smalls", bufs=6)   # per-row stats
        diagp = tc.alloc_tile_pool(name="diagp", bufs=10)     # diag(1/r) tiles
        ppool = tc.alloc_tile_pool(name="ppool", bufs=7)     # softmax P rows

        dummy_mms(20)
        nc.sync.dma_start(mask_sb[:], maskd.ap())
        nc.sync.dma_start(ident_sb[:], identd.ap())
        xbpool = tc.alloc_tile_pool(name="xbpool", bufs=1)   # bf16 x^T (V proj)
        xbf_sb = xbpool.tile([128, KO, S], BF16)

        # ---------------- Phase 1: Q/K projections ----------------
        xpool = tc.alloc_tile_pool(name="xpool", bufs=1)
        xT_sb = xpool.tile([128, KO, S], F32R)
        wqk = tc.alloc_tile_pool(name="wqk", bufs=3)

        # Front DMA order: wq0, x sc0-half, wq1, wq2, x sc1-half, then JIT.
        wq_tiles = {}
        wq_tiles[0] = wqk.tile([128, KO, 128], F32R, tag="wqk", name="wqe")
        nc.sync.dma_start(wq_tiles[0][:], wqt_r[:, 0])
        for kc in range(4):
            nc.sync.dma_start(
                xT_sb[:, 4 * kc:4 * (kc + 1), 0:512],
                xt_r[:, 0, 4 * kc:4 * (kc + 1), :],
            )
        for e in (1, 2):
            wq_tiles[e] = wqk.tile([128, KO, 128], F32R, tag="wqk", name="wqe")
            nc.sync.dma_start(wq_tiles[e][:], wqt_r[:, e])
        # second s-half of x^T queues behind the first three weight tiles so
        # the early projection groups never wait on weights; it still lands
        # just before the first sc=1 group needs it
        for kc in range(4):
            nc.sync.dma_start(
                xT_sb[:, 4 * kc:4 * (kc + 1), 512:1024],
                xt_r[:, 1, 4 * kc:4 * (kc + 1), :],
            )

        def proj_group(wt, dst, e, sc):
            ps = pp.tile([128, 512], F32, tag="pp")
            for k in range(KO):
                nc.tensor.matmul(
                    ps[:],
                    wt[:, k, :],
                    xT_sb[:, k, sc * 512:(sc + 1) * 512],
                    start=(k == 0),
                    stop=(k == KO - 1),
                )
            nc.scalar.copy(dst[:, e, sc * 512:(sc + 1) * 512], ps[:])

        # Q pass: interleaved group order, 3-tile weight window with a
        # two-group DMA lookahead.
        for idx, (e, sc) in enumerate(QGROUPS):
            for la in (0, 1, 2):
                if idx + la < len(QGROUPS):
                    ela = QGROUPS[idx + la][0]
                    if ela not in wq_tiles:
                        wq_tiles[ela] = wqk.tile(
                            [128, KO, 128], F32R, tag="wqk", name="wqe")
                        nc.sync.dma_start(wq_tiles[ela][:], wqt_r[:, ela])
            proj_group(wq_tiles[e], qt_sb, e, sc)
            if sc == 1:
                del wq_tiles[e]  # last use; slot recycles

        # K pass: e-outer (x^T fully resident), streamed weight tiles; the
        # bf16 x^T for the V projection streams between the weight tiles.
        for e in range(HL):
            wt = wqk.tile([128, KO, 128], F32R, tag="wqk")
            nc.sync.dma_start(wt[:], wkt_r[:, e])
            for sc in range(2):
                proj_group(wt, kt_sb, e, sc)
            if e % 2 == 1:
                kc = e // 2
                nc.sync.dma_start(
                    xbf_sb[:, 4 * kc:4 * (kc + 1), :],
                    xtbf_r[:, 4 * kc:4 * (kc + 1), :],
                )
        wqk.release()
        xpool.release()

        # ---------------- Phase 2: attention + V/out-proj fillers ----------
        # These pools reuse the released xT/wqk address range.
        v_sb_pool = tc.alloc_tile_pool(name="vsb", bufs=1)
        v_sb = v_sb_pool.tile([128, ST, E], BF16)    # V: [s_in, s_out, e]
        ptpool = tc.alloc_tile_pool(name="ptpool", bufs=5)   # transposed P
        ctxpp = tc.alloc_tile_pool(name="ctxpp", bufs=3)     # per-pair ctx^T
        stage = tc.alloc_tile_pool(name="stage", bufs=2)     # out staging
        wvpool = tc.alloc_tile_pool(name="wvpool", bufs=1)
        wv_sb = wvpool.tile([128, KO, E], BF16)
        for kg in range(4):
            nc.sync.dma_start(
                wv_sb[:, 4 * kg:4 * (kg + 1), :], wvt_r[:, 4 * kg:4 * (kg + 1), :]
            )

        def vproj_stile(si):
            """bf16 V projection for one s-tile (dense PE filler)."""
            for ec in range(2):
                ps = pp.tile([128, 512], F32, tag="pp")
                for k in range(KO):
                    nc.tensor.matmul(
                        ps[:],
                        xbf_sb[:, k, si * 128:(si + 1) * 128],
                        wv_sb[:, k, ec * 512:(ec + 1) * 512],
                        start=(k == 0),
                        stop=(k == KO - 1),
                    )
                nc.scalar.copy(v_sb[:, si, ec * 512:(ec + 1) * 512], ps[:])

        def scores_softmax(h, t):
            """Masked scaled softmax row block for (head h, q-tile t).

            Returns (p_sb, diag): unnormalized exp rows plus the bf16
            diag(1/rowsum) tile that the P-transpose streams to normalize.
            """
            width = (t + 1) * 128
            c0w = min(width, 512)
            c1w = width - c0w
            qs = qt_sb[:, h, t * 128:(t + 1) * 128]
            ps0 = ps_s.tile([128, 512], F32, tag="ps_s")
            nc.tensor.matmul(
                ps0[:, :c0w], qs, kt_sb[:, h, 0:c0w], start=True, stop=True
            )
            ps1 = None
            if c1w:
                ps1 = ps_s.tile([128, 512], F32, tag="ps_s")
                nc.tensor.matmul(
                    ps1[:, :c1w], qs, kt_sb[:, h, 512:512 + c1w],
                    start=True, stop=True,
                )
            # causal mask on the diagonal 128x128 block
            if t < 4:
                diag_blk = ps0[:, t * 128:(t + 1) * 128]
            else:
                diag_blk = ps1[:, (t - 4) * 128:(t - 3) * 128]
            nc.vector.tensor_add(diag_blk, diag_blk, mask_sb[:])

            nm = smalls.tile([128, 1], F32, tag="nm")
            nc.vector.reduce_max(
                nm[:], ps0[:, :c0w], axis=mybir.AxisListType.X, negate=True
            )
            if c1w:
                nm1 = smalls.tile([128, 1], F32, tag="nm1")
                nc.vector.reduce_max(
                    nm1[:], ps1[:, :c1w], axis=mybir.AxisListType.X, negate=True
                )
                nc.vector.tensor_tensor(nm[:], nm[:], nm1[:], mybir.AluOpType.min)
            bias = smalls.tile([128, 1], F32, tag="bias")
            nc.vector.tensor_scalar_mul(bias[:], nm[:], SCALE)

            p_sb = ppool.tile([128, width], BF16, tag="p")
            r0 = smalls.tile([128, 1], F32, tag="r0")
            nc.scalar.activation(
                p_sb[:, :c0w], ps0[:, :c0w], mybir.ActivationFunctionType.Exp,
                bias=bias[:], scale=SCALE, accum_out=r0[:],
            )
            if c1w:
                r1 = smalls.tile([128, 1], F32, tag="r1")
                nc.scalar.activation(
                    p_sb[:, 512:512 + c1w], ps1[:, :c1w],
                    mybir.ActivationFunctionType.Exp,
                    bias=bias[:], scale=SCALE, accum_out=r1[:],
                )
                nc.vector.tensor_add(r0[:], r0[:], r1[:])
            rr = smalls.tile([128, 1], F32, tag="rr")
            nc.vector.reciprocal(rr[:], r0[:])
            diag = diagp.tile([128, 128], BF16, tag="diag")
            nc.vector.tensor_scalar_mul(diag[:], ident_sb[:], rr[:])
            return p_sb, diag

        def transposes(tp, h, pe_, de, po_, do):
            """Normalizing P-transposes (regular matmuls vs diag(1/r))."""
            nk = 2 * tp + 2
            pts = []
            for j0 in range(0, nk, 2):
                pt_ps = ps_t.tile([128, 512], F32, tag="ps_t")
                pt_sb = ptpool.tile([128, 512], BF16, tag="pt")
                # alternate the PSUM->SBUF drains between ACT and DVE so
                # neither engine's queue stalls the PE transposes
                cp = nc.scalar.copy if (j0 // 2) % 2 == 0 else nc.vector.tensor_copy
                # even-tile blocks first: the odd tile's softmax finishes
                # later, so its diag is consumed as late as possible
                for dj in (0, 1):
                    j = j0 + dj
                    col = dj * 256
                    if j < nk - 1:
                        nc.tensor.matmul(
                            pt_ps[:, col:col + 128],
                            pe_[:, j * 128:(j + 1) * 128], de[:],
                            start=True, stop=True,
                        )
                for dj in (0, 1):
                    j = j0 + dj
                    col = dj * 256
                    nc.tensor.matmul(
                        pt_ps[:, col + 128:col + 256],
                        po_[:, j * 128:(j + 1) * 128], do[:],
                        start=True, stop=True,
                    )
                if j0 + 1 == nk - 1:
                    cp(pt_sb[:, 0:256], pt_ps[:, 0:256])
                    cp(pt_sb[:, 384:512], pt_ps[:, 384:512])
                else:
                    cp(pt_sb[:], pt_ps[:])
                pts.append(pt_sb)
            return pts

        def pv(tp, h, pts, ctx_pair):
            """PV accumulation for head h, pair tp from transposed P tiles."""
            nk = 2 * tp + 2
            ctx_ps = ps_c.tile([128, 256], F32, tag="ps_c")
            for j0 in range(0, nk, 2):
                pt_sb = pts[j0 // 2]
                for dj in (0, 1):
                    j = j0 + dj
                    col = dj * 256
                    vt = v_sb[:, j, h * 128:(h + 1) * 128]
                    if j < nk - 1:
                        nc.tensor.matmul(
                            ctx_ps[:], vt, pt_sb[:, col:col + 256],
                            start=(j == 0), stop=(j == nk - 1),
                        )
                    else:
                        nc.tensor.matmul(
                            ctx_ps[:, 128:256], vt,
                            pt_sb[:, col + 128:col + 256],
                            start=(j == 0), stop=True,
                        )
            nc.scalar.copy(ctx_pair[:, h, :], ctx_ps[:])

        def pair_rolling(tp, ctx_pair, filler=None, pre=None):
            """One q-tile pair: even-tile scores 3 heads ahead, odd 2 ahead."""
            te, to = 2 * tp, 2 * tp + 1
            sme = {}
            smo = {}
            for h in range(3):
                sme[h] = scores_softmax(h, te)
            for h in range(2):
                smo[h] = scores_softmax(h, to)
            if pre is not None:
                pre()
            for h in range(HL):
                if h + 3 < HL:
                    sme[h + 3] = scores_softmax(h + 3, te)
                # out-proj / V-proj fillers run before the transposes so the
                # just-issued score tiles get maximal softmax-chain cover
                if filler is not None:
                    filler(h)
                pe_, de = sme.pop(h)
                po_, do = smo.pop(h)
                pts = transposes(tp, h, pe_, de, po_, do)
                if h + 2 < HL:
                    smo[h + 2] = scores_softmax(h + 2, to)
                pv(tp, h, pts, ctx_pair)

        wo_tiles = []

        def outproj_chunk(ctx_tile, tp, chunk):
            """One of the 8 output-projection chunks for finished pair tp."""
            sl, oc = divmod(chunk, 4)
            si = 2 * tp + sl
            ps = pp.tile([128, 512], F32, tag="pp")
            for j in range(HL):
                nc.tensor.matmul(
                    ps[:],
                    ctx_tile[:, j, sl * 128:(sl + 1) * 128],
                    wo_tiles[oc][:, j, :],
                    start=(j == 0),
                    stop=(j == HL - 1),
                )
            ob = stage.tile([128, 512], F32, tag="ob")
            nc.scalar.copy(ob[:], ps[:])
            nc.sync.dma_start(out_r[:, si, oc * 512:(oc + 1) * 512], ob[:])

        # pair 0: dummies + the first two V s-tiles bridge the wv DMA; the
        # remaining V s-tiles ride along as fillers through pairs 0/1.
        ctx0 = ctxpp.tile([128, HL, 256], BF16, tag="ctx")

        def pre0():
            dummy_mms(22)
            vproj_stile(0)
            vproj_stile(1)

        def fill0(h):
            if h == 2:
                vproj_stile(2)
            elif h == 5:
                vproj_stile(3)

        pair_rolling(0, ctx0, filler=fill0, pre=pre0)

        ctx1 = ctxpp.tile([128, HL, 256], BF16, tag="ctx")

        def fill1(h):
            if h % 2 == 1:
                vproj_stile(4 + h // 2)

        pair_rolling(1, ctx1, filler=fill1)

        def swap_wv_for_wo():
            # after pair-2's score prefetch: drop wv, load Wo resident
            wvpool.release()
            wo = tc.alloc_tile_pool(name="wo", bufs=4)
            for oc in range(4):
                w = wo.tile([128, HL, 512], BF16, tag="wo", name="wot_t")
                nc.sync.dma_start(w[:], wot_r[:, :, oc * 512:(oc + 1) * 512])
                wo_tiles.append(w)
            wo_pool_holder.append(wo)

        wo_pool_holder = []
        ctx2 = ctxpp.tile([128, HL, 256], BF16, tag="ctx")
        pair_rolling(2, ctx2, filler=lambda h: (outproj_chunk(ctx0, 0, h),
                                                outproj_chunk(ctx1, 1, h)),
                     pre=swap_wv_for_wo)
        ctx3 = ctxpp.tile([128, HL, 256], BF16, tag="ctx")
        pair_rolling(3, ctx3, filler=lambda h: outproj_chunk(ctx2, 2, h))
        for chunk in range(8):
            outproj_chunk(ctx3, 3, chunk)

        wo_pool_holder[0].release()
        stage.release()
        ctxpp.release()
        ptpool.release()
        v_sb_pool.release()
        xbpool.release()
        ppool.release()
        diagp.release()
        smalls.release()
        persist.release()
        ps_c.release()
        ps_t.release()
        ps_s.release()
        pp.release()

    nc.compile()
    return nc


_NC = None


def _get_nc():
    global _NC
    if _NC is None:
        _NC = build_nc()
    return _NC


def _make_in_maps(x, Wq, Wk, Wv, Wo):
    x = np.asarray(x, dtype=np.float32)
    Wq = np.asarray(Wq, dtype=np.float32)
    Wk = np.asarray(Wk, dtype=np.float32)
    Wv = np.asarray(Wv, dtype=np.float32)
    Wo = np.asarray(Wo, dtype=np.float32)

    import ml_dtypes
    mask = np.triu(np.full((128, 128), NEG, dtype=np.float32), k=1)
    ident = np.eye(128, dtype=ml_dtypes.bfloat16)

    def _tile_x(xb):
        # [D, S] -> [sc, p, ko, c] with 8 KB contiguous per (sc, p, 4ko) chunk
        t = xb.T.reshape(KO, 128, 2, 512)
        return np.ascontiguousarray(t.transpose(2, 1, 0, 3))

    xts = [_tile_x(x[b]) for b in range(B)]
    xts_plain = [np.ascontiguousarray(x[b].T) for b in range(B)]
    xtbfs = [t.astype(ml_dtypes.bfloat16) for t in xts_plain]
    def _tile_w(Wm, hh):
        # [D, E] slice^T -> [e-tile, p, ko, c] with 8 KB contiguous per (e, p)
        wt = Wm[hh * E:(hh + 1) * E, :].T          # [D, E]
        wt = wt.reshape(KO, 128, HL, 128)           # [ko, p, e, c]
        return np.ascontiguousarray(wt.transpose(2, 1, 0, 3))  # [e, p, ko, c]

    wqts = [_tile_w(Wq, hh) for hh in range(2)]
    wkts = [_tile_w(Wk, hh) for hh in range(2)]
    wvts = [np.ascontiguousarray(Wv[hh * E:(hh + 1) * E, :].T).astype(ml_dtypes.bfloat16)
            for hh in range(2)]
    wots = [np.ascontiguousarray(Wo[:, hh * E:(hh + 1) * E].T).astype(ml_dtypes.bfloat16)
            for hh in range(2)]

    in_maps = []
    for b in range(B):
        for hh in range(2):
            in_maps.append({
                "xt": xts[b],
                "xtbf": xtbfs[b],
                "wqt": wqts[hh],
                "wkt": wkts[hh],
                "wvt": wvts[hh],
                "wot": wots[hh],
                "maskd": mask,
                "identd": ident,
            })
    return in_maps


def run(x, Wq, Wk, Wv, Wo, **rb_kwargs):
    """Run on 8 cores; returns (output [B,S,D], BassKernelResults)."""
    nc = _get_nc()
    in_maps = _make_in_maps(x, Wq, Wk, Wv, Wo)
    res = run_bass_kernel_spmd(nc, in_maps, core_ids=list(range(8)), **rb_kwargs)
    out = np.empty((B, S, D), dtype=np.float32)
    for b in range(B):
        out[b] = res.results[2 * b]["out"] + res.results[2 * b + 1]["out"]
    return out, res


def kernel(x, Wq, Wk, Wv, Wo):
    out, _ = run(x, Wq, Wk, Wv, Wo)
    return out
